# revision 3
# baseline (speedup 1.0000x reference)
"""Trainium2 Bass kernel for NNBlendFM: 3-layer tanh MLP embedder + 64-head
rank-16 factorization machine, data-parallel over batch across 8 NeuronCores.

Math (per batch row b, head h):
    h = tanh(tanh(tanh(x W1 + b1) W2 + b2) W3 + b3)          # [B, 2048]
    lin[b,h]  = h . fm_w[h]
    vx[b,h,r] = h . fm_V[h,r]
    diag[b,h] = (h*h) . (sum_r fm_V[h,r]^2)
    out[h,b]  = fm_w0[h] + lin + 0.5*(sum_r vx^2 - diag)

Device layout: activations kept as [feature_partition, batch_free] tiles so
every matmul contracts over the partition dim with natural-layout weights as
the stationary operand.  The FM stage flips to [batch_partition, col_free] by
using h^T k-tiles as the stationary operand.  All matmul inputs are bf16
(fp32 PSUM accumulation), everything else fp32.

Schedule notes (v2):
  * All weights are host-packed into [128, k*cols] row-major order so each
    SBUF tile fills with ONE large contiguous DMA (k-pair tiles of 8 KiB per
    partition for W2/W3) -- DMA issue on a sequencer costs ~0.7 us, so issue
    count is minimized everywhere except the critical head.
  * Head: L1 runs batch-chunk-outer (c0 all jt, then c1), so it can start
    once HALF of x and HALF of W1's columns have landed.  Those four 256 KiB
    + four 128 KiB wave-1 DMAs are fanned across all four non-PE sequencers
    and land ~11.5 us in; warm-up matmuls on a memset tile bridge the gap so
    the PE HAM throttle stays released (idle >3.4 us would re-throttle).
  * W3 pair-tiles deliberately overflow the weight pool ring: the first four
    land early (spare + freed-W1 slots), the last four wait for W2 slots that
    free during L2's final jt group and land just before L3 reads them.  VT
    similarly streams into freed W2 slots during L3.
  * Output is accumulated into one [128, 8*64] tile and shipped with a
    single DMA at the end (one issue instead of eight).
"""

import numpy as np
import ml_dtypes

import concourse.tile as tile
from concourse import bacc, mybir
from concourse import bass_utils

BF16 = mybir.dt.bfloat16
F32 = mybir.dt.float32
AF = mybir.ActivationFunctionType
ALU = mybir.AluOpType

P = 128
IN, HID, HEADS, RANK = 512, 2048, 64, 16
B = 8192
NCORES = 8
BC = B // NCORES            # 1024 batch rows per core
KT1 = IN // P               # 4  k-tiles, layer 1
KT = HID // P               # 16 k-tiles, layers 2/3 + FM
JT = HID // P               # 16 output-feature tiles per layer
NB = 512                    # matmul moving free-dim (one PSUM bank)
NBC = BC // NB              # 2 batch column chunks
BT = BC // P                # 8 batch tiles in FM stage
HR = HEADS * RANK           # 1024 vx columns
WARMUP_MM = 14              # ~3.4us cold + ~1.1us warm of PE busy

_CACHE = {}


def _build_module():
    nc = bacc.Bacc(
        "TRN2", target_bir_lowering=False, debug=False, num_devices=NCORES
    )
    dt = nc.dram_tensor
    # host-packed layouts: [p, k*cols + c] = M[k*128 + p, c]
    XP = dt("XP", [P, KT1 * BC], BF16, kind="ExternalInput").ap()
    W1P = dt("W1P", [P, KT1 * HID], BF16, kind="ExternalInput").ap()
    W2P = dt("W2P", [P, KT * HID], BF16, kind="ExternalInput").ap()
    W3P = dt("W3P", [P, KT * HID], BF16, kind="ExternalInput").ap()
    VTP = dt("VTP", [P, KT * HR], BF16, kind="ExternalInput").ap()
    B1 = dt("B1", [P, JT], F32, kind="ExternalInput").ap()
    B2 = dt("B2", [P, JT], F32, kind="ExternalInput").ap()
    B3 = dt("B3", [P, JT], F32, kind="ExternalInput").ap()
    FW = dt("FW", [P, KT * HEADS], BF16, kind="ExternalInput").ap()
    SQ = dt("SQ", [P, KT * HEADS], BF16, kind="ExternalInput").ap()
    W0C = dt("W0C", [P, HEADS], BF16, kind="ExternalInput").ap()
    OUT = dt("out", [BC, HEADS], F32, kind="ExternalOutput").ap()

    with tile.TileContext(nc) as tc:
        with (
            tc.tile_pool(name="wpool", bufs=12) as wpool,   # 12 x 8KiB
            tc.tile_pool(name="hpool", bufs=32) as hpool,   # 32 x 2KiB
            tc.tile_pool(name="cpool", bufs=1) as cpool,
            tc.tile_pool(name="pp", bufs=8, space="PSUM") as pp,
            tc.tile_pool(name="epool", bufs=2) as epool,
            tc.tile_pool(name="spool", bufs=8) as spool,
            tc.tile_pool(name="opool", bufs=1) as opool,
        ):
            # --- PE warm-up --------------------------------------------------
            # Dummy matmuls on a memset tile keep the PE busy through the DMA
            # head so HAM un-throttles (1.2 -> 2.4 GHz) and STAYS released
            # until the first real matmul's inputs land (~11.5us).
            warm = cpool.tile([P, NB], BF16, tag="warm")
            nc.vector.memset(warm[:], 0.0)
            wu = pp.tile([P, NB], F32, tag="ps", name="warm")
            for _ in range(WARMUP_MM):
                nc.tensor.matmul(
                    wu[:], warm[:, 0:P], warm[:], start=True, stop=True
                )

            # --- SBUF tiles (allocation order defines pool-ring reuse) -------
            xt = [hpool.tile([P, BC], BF16, tag="h", name=f"xt{k}")
                  for k in range(KT1)]
            w1p = [wpool.tile([P, 2 * HID], BF16, tag="w", name=f"w1p{k}")
                   for k in range(KT1 // 2)]
            w2p = [wpool.tile([P, 2 * HID], BF16, tag="w", name=f"w2p{k}")
                   for k in range(KT // 2)]
            w3p = [wpool.tile([P, 2 * HID], BF16, tag="w", name=f"w3p{k}")
                   for k in range(KT // 2)]
            vtq = [wpool.tile([P, 4 * HR], BF16, tag="w", name=f"vtq{q}")
                   for q in range(KT // 4)]

            def wsl(tiles, kt, j):
                """[128,128] stationary slice for k-tile kt, out-feature j."""
                return tiles[kt // 2][:, (kt % 2) * HID + j * P: (kt % 2) * HID + (j + 1) * P]

            # --- DMA issue plan ---------------------------------------------
            # Only sync/gpsimd/scalar sequencers can issue DMAs (~0.7us per
            # issue).  wave 1 (critical): x batch-half c0 + W1 column-half
            # h0, fanned so all eight are in flight by ~8us.
            def w1_dma(k, h, eng):
                eng.dma_start(
                    w1p[k // 2][:, (k % 2) * HID + h * (HID // 2):
                                (k % 2) * HID + (h + 1) * (HID // 2)],
                    W1P[:, k * HID + h * (HID // 2): k * HID + (h + 1) * (HID // 2)],
                )

            nc.scalar.dma_start(xt[0][:, 0:NB], XP[:, 0 * BC: 0 * BC + NB])
            nc.sync.dma_start(xt[1][:, 0:NB], XP[:, 1 * BC: 1 * BC + NB])
            nc.gpsimd.dma_start(xt[2][:, 0:NB], XP[:, 2 * BC: 2 * BC + NB])
            w1_dma(0, 0, nc.scalar)
            w1_dma(1, 0, nc.sync)
            w1_dma(2, 0, nc.gpsimd)
            nc.scalar.dma_start(xt[3][:, 0:NB], XP[:, 3 * BC: 3 * BC + NB])
            w1_dma(3, 0, nc.sync)
            # wave 2: x c1 + W1 h1
            nc.gpsimd.dma_start(xt[0][:, NB:BC], XP[:, 0 * BC + NB: 1 * BC])
            w1_dma(0, 1, nc.scalar)
            nc.sync.dma_start(xt[1][:, NB:BC], XP[:, 1 * BC + NB: 2 * BC])
            nc.gpsimd.dma_start(xt[2][:, NB:BC], XP[:, 2 * BC + NB: 3 * BC])
            w1_dma(1, 1, nc.scalar)
            w1_dma(2, 1, nc.sync)
            nc.gpsimd.dma_start(xt[3][:, NB:BC], XP[:, 3 * BC + NB: 4 * BC])
            w1_dma(3, 1, nc.sync)

            # small constants on gpsimd (b1 needed by first ACT ~13us)
            b1t = cpool.tile([P, JT], F32, tag="b1")
            nc.gpsimd.dma_start(b1t[:], B1)
            b2t = cpool.tile([P, JT], F32, tag="b2")
            nc.gpsimd.dma_start(b2t[:], B2)
            b3t = cpool.tile([P, JT], F32, tag="b3")
            nc.gpsimd.dma_start(b3t[:], B3)
            # -w0/128 replicated; contracted against a ones column block so
            # the diag PSUM group finishes as (0.5*diag - w0).
            w0c = cpool.tile([P, HEADS], BF16, tag="w0c")
            nc.gpsimd.dma_start(w0c[:], W0C)
            onest = cpool.tile([P, P], BF16, tag="ones")
            nc.gpsimd.memset(onest[:], 1.0)
            fwt = cpool.tile([P, KT * HEADS], BF16, tag="fw")
            nc.sync.dma_start(fwt[:], FW)
            sqt = cpool.tile([P, KT * HEADS], BF16, tag="sq")
            nc.gpsimd.dma_start(sqt[:], SQ)

            # bulk weights: one DMA per 8KiB-per-partition pair tile.
            # w3p[2..7] and vtq[*] intentionally wait for ring slots that
            # free at L1-end / during L2's last group -- see module docstring.
            for j in range(4):
                nc.sync.dma_start(w2p[j][:], W2P[:, j * 2 * HID: (j + 1) * 2 * HID])
            for j in range(4, 8):
                nc.gpsimd.dma_start(w2p[j][:], W2P[:, j * 2 * HID: (j + 1) * 2 * HID])
            for j in range(4):
                nc.sync.dma_start(w3p[j][:], W3P[:, j * 2 * HID: (j + 1) * 2 * HID])
            for j in range(4, 8):
                nc.gpsimd.dma_start(w3p[j][:], W3P[:, j * 2 * HID: (j + 1) * 2 * HID])
            nc.sync.dma_start(vtq[0][:], VTP[:, 0 * 4 * HR: 1 * 4 * HR])
            nc.sync.dma_start(vtq[1][:], VTP[:, 1 * 4 * HR: 2 * 4 * HR])
            nc.gpsimd.dma_start(vtq[2][:], VTP[:, 2 * 4 * HR: 3 * 4 * HR])
            nc.gpsimd.dma_start(vtq[3][:], VTP[:, 3 * 4 * HR: 4 * 4 * HR])

            # --- embedder ----------------------------------------------------
            def layer1():
                """c-outer so jt 0-7 of chunk c0 only need wave-1 data."""
                h_out = [hpool.tile([P, BC], BF16, tag="h", name=f"l1h{j}")
                         for j in range(JT)]
                for c in range(NBC):
                    for jt in range(JT):
                        ps = pp.tile([P, NB], F32, tag="ps", name=f"l1ps{c}_{jt}")
                        kts = [(kt + jt) % KT1 for kt in range(KT1)]
                        for i, kt in enumerate(kts):
                            nc.tensor.matmul(
                                ps[:],
                                wsl(w1p, kt, jt),
                                xt[kt][:, c * NB: (c + 1) * NB],
                                start=(i == 0),
                                stop=(i == KT1 - 1),
                            )
                        nc.scalar.activation(
                            h_out[jt][:, c * NB: (c + 1) * NB],
                            ps[:],
                            AF.Tanh,
                            bias=b1t[:, jt: jt + 1],
                        )
                return h_out

            def layer(h_prev, w_pairs, bias_t, name):
                h_out = []
                for jt in range(JT):
                    ps = [pp.tile([P, NB], F32, tag="ps", name=f"{name}ps{jt}_{c}")
                          for c in range(NBC)]
                    # Rotate the accumulation order by jt so each weight
                    # tile's final read retires early for some jt, releasing
                    # its pool slot for the next layer's prefetch DMA.
                    kts = [(kt + jt) % KT for kt in range(KT)]
                    for i, kt in enumerate(kts):
                        lhsT = wsl(w_pairs, kt, jt)
                        for c in range(NBC):
                            nc.tensor.matmul(
                                ps[c][:],
                                lhsT,
                                h_prev[kt][:, c * NB: (c + 1) * NB],
                                start=(i == 0),
                                stop=(i == KT - 1),
                            )
                    ht = hpool.tile([P, BC], BF16, tag="h", name=f"{name}h{jt}")
                    for c in range(NBC):
                        nc.scalar.activation(
                            ht[:, c * NB: (c + 1) * NB],
                            ps[c][:],
                            AF.Tanh,
                            bias=bias_t[:, jt: jt + 1],
                        )
                    h_out.append(ht)
                return h_out

            h1 = layer1()
            h2 = layer(h1, w2p, b2t, "l2")
            h3 = layer(h2, w3p, b3t, "l3")

            # --- h3 squared (stationary operand for the diag matmuls) -----
            h3sq = []
            for k in range(KT):
                sq_k = hpool.tile([P, BC], BF16, tag="h", name=f"h3sq{k}")
                nc.vector.tensor_mul(sq_k[:], h3[k][:], h3[k][:])
                h3sq.append(sq_k)

            # --- FM stage: per 128-row batch tile -------------------------
            def vsl(kt, half):
                """[128,512] moving slice of V^T for k-tile kt."""
                base = (kt % 4) * HR + half * NB
                return vtq[kt // 4][:, base: base + NB]

            def fm_phase_a(bt):
                """vx = h V^T (1024 cols) and lin = h fm_w^T (64 cols)."""
                vx0 = pp.tile([P, NB], F32, tag="ps", name=f"vx0_{bt}")
                vx1 = pp.tile([P, NB], F32, tag="ps", name=f"vx1_{bt}")
                lw = pp.tile([P, NB], F32, tag="ps", name=f"lw_{bt}")
                bsl = slice(bt * P, (bt + 1) * P)
                for kt in range(KT):
                    lhsT = h3[kt][:, bsl]
                    nc.tensor.matmul(
                        vx0[:], lhsT, vsl(kt, 0),
                        start=(kt == 0), stop=(kt == KT - 1),
                    )
                    nc.tensor.matmul(
                        vx1[:], lhsT, vsl(kt, 1),
                        start=(kt == 0), stop=(kt == KT - 1),
                    )
                    nc.tensor.matmul(
                        lw[:, 0:HEADS], lhsT,
                        fwt[:, kt * HEADS: (kt + 1) * HEADS],
                        start=(kt == 0), stop=(kt == KT - 1),
                    )
                return vx0, vx1, lw

            def fm_phase_b(bt):
                """diag = (h*h) . (0.5 * sum_r V^2), already scaled by 0.5."""
                dg = pp.tile([P, NB], F32, tag="ps", name=f"dg_{bt}")
                bsl = slice(bt * P, (bt + 1) * P)
                for kt in range(KT):
                    nc.tensor.matmul(
                        dg[:, 0:HEADS],
                        h3sq[kt][:, bsl],
                        sqt[:, kt * HEADS: (kt + 1) * HEADS],
                        start=(kt == 0), stop=False,
                    )
                nc.tensor.matmul(
                    dg[:, 0:HEADS], onest[:], w0c[:], start=False, stop=True,
                )
                return dg

            def fm_square_reduce(bt, vx0, vx1):
                """Emitted right after phase A: overlaps later bt's matmuls.
                Each 512-wide half squares then reduces independently so the
                two chains pipeline across ACT and DVE."""
                vx2 = epool.tile([P, HR], F32, tag="e", name=f"vx2_{bt}")
                sumv = spool.tile([P, HEADS], F32, tag="s", name=f"sumv_{bt}")
                for c, vxh in ((0, vx0), (1, vx1)):
                    nc.scalar.activation(vx2[:, c * NB: (c + 1) * NB], vxh[:], AF.Square)
                    nc.vector.reduce_sum(
                        sumv[:, c * (HEADS // 2): (c + 1) * (HEADS // 2)],
                        vx2[:, c * NB: (c + 1) * NB].rearrange(
                            "p (h r) -> p h r", r=RANK
                        ),
                        axis=mybir.AxisListType.X,
                    )
                return sumv

            ot = opool.tile([P, BT * HEADS], F32, tag="o")

            def fm_combine(bt, sumv, lw, dg):
                # q = 0.5*sumv - diag_half
                q = spool.tile([P, HEADS], F32, tag="s", name=f"q_{bt}")
                nc.vector.scalar_tensor_tensor(
                    q[:], sumv[:], 0.5, dg[:, 0:HEADS],
                    op0=ALU.mult, op1=ALU.subtract,
                )
                nc.vector.tensor_add(
                    ot[:, bt * HEADS: (bt + 1) * HEADS], q[:], lw[:, 0:HEADS]
                )

            # Stagger: A(0), A(1), B(0), C(0), A(2), B(1), C(1), ...
            pend = []  # (bt, sumv, lw)
            for bt in range(BT):
                vx0, vx1, lw = fm_phase_a(bt)
                sumv = fm_square_reduce(bt, vx0, vx1)
                pend.append((bt, sumv, lw))
                if len(pend) == 2:
                    obt, osumv, olw = pend.pop(0)
                    dg = fm_phase_b(obt)
                    fm_combine(obt, osumv, olw, dg)
            while pend:
                obt, osumv, olw = pend.pop(0)
                dg = fm_phase_b(obt)
                fm_combine(obt, osumv, olw, dg)

            # one DMA for the whole per-core output: dram row bt*128+p.
            nc.gpsimd.dma_start(
                OUT.rearrange("(bt p) c -> p bt c", bt=BT), ot[:]
            )

    nc.compile()
    return nc


def _get_nc():
    if "nc" not in _CACHE:
        _CACHE["nc"] = _build_module()
    return _CACHE["nc"]


def _pack_rows(M, kt):
    """[kt*128, C] -> [128, kt*C] with [p, k*C+c] = M[k*128+p, c]."""
    C = M.shape[1]
    return np.ascontiguousarray(
        M.reshape(kt, P, C).transpose(1, 0, 2).reshape(P, kt * C)
    )


def _prep_host(x, W1, b1, W2, b2, W3, b3, fm_w0, fm_w, fm_V):
    """Host-side layout prep: bf16 casts, packing, per-head V reductions."""
    bf = ml_dtypes.bfloat16
    f32 = np.float32

    common = {
        "W1P": _pack_rows(W1.astype(bf), KT1),
        "W2P": _pack_rows(W2.astype(bf), KT),
        "W3P": _pack_rows(W3.astype(bf), KT),
        "B1": np.ascontiguousarray(b1.astype(f32).reshape(JT, P).T),
        "B2": np.ascontiguousarray(b2.astype(f32).reshape(JT, P).T),
        "B3": np.ascontiguousarray(b3.astype(f32).reshape(JT, P).T),
        # V^T: [2048, heads*rank] packed as [128, 16*1024]
        "VTP": _pack_rows(
            fm_V.reshape(HEADS * RANK, HID).T.astype(bf), KT
        ),
        # fm_w^T packed as [128, kt*64]: FW[p, kt*64+h] = fm_w[h, kt*128+p]
        "FW": np.ascontiguousarray(
            fm_w.T.reshape(KT, P, HEADS).transpose(1, 0, 2).reshape(P, KT * HEADS)
            .astype(bf)
        ),
        # 0.5 * sum_r V^2, same packing
        "SQ": np.ascontiguousarray(
            (0.5 * (fm_V.astype(np.float64) ** 2).sum(axis=1))
            .T.reshape(KT, P, HEADS).transpose(1, 0, 2).reshape(P, KT * HEADS)
            .astype(bf)
        ),
        "W0C": np.ascontiguousarray(
            np.tile((-fm_w0.astype(np.float64) / P)[None, :], (P, 1))
            .astype(ml_dtypes.bfloat16)
        ),
    }

    in_maps = []
    xb = x.astype(bf)
    for c in range(NCORES):
        m = dict(common)
        m["XP"] = _pack_rows(
            np.ascontiguousarray(xb[c * BC: (c + 1) * BC, :].T), KT1
        )
        in_maps.append(m)
    return in_maps


def kernel(x, W1, b1, W2, b2, W3, b3, fm_w0, fm_w, fm_V):
    # Host prep is plain numpy; coerce eagerly in case inputs are jax arrays.
    x, W1, b1, W2, b2, W3, b3, fm_w0, fm_w, fm_V = (
        np.asarray(a) for a in (x, W1, b1, W2, b2, W3, b3, fm_w0, fm_w, fm_V)
    )
    nc = _get_nc()
    in_maps = _prep_host(x, W1, b1, W2, b2, W3, b3, fm_w0, fm_w, fm_V)
    import os
    trace = bool(int(os.environ.get("KERNEL_TRACE", "0")))
    last_err = None
    for _attempt in range(3):
        try:
            res = bass_utils.run_bass_kernel_spmd(
                nc, in_maps, core_ids=list(range(NCORES)), trace=trace,
            )
            outs = [np.asarray(res.results[c]["out"]) for c in range(NCORES)]
            break
        except Exception as e:  # transient device faults (NRT unrecoverable)
            last_err = e
    else:
        raise last_err
    _CACHE["last_results"] = res
    full = np.concatenate(outs, axis=0)          # [B, HEADS]
    return np.ascontiguousarray(full.T).astype(np.float32)  # [HEADS, B]


# revision 4
# speedup vs baseline: 1.0489x; 1.0489x over previous
"""Trainium2 Bass kernel for NNBlendFM: 3-layer tanh MLP embedder + 64-head
rank-16 factorization machine, data-parallel over batch across 8 NeuronCores.

Math (per batch row b, head h):
    h = tanh(tanh(tanh(x W1 + b1) W2 + b2) W3 + b3)          # [B, 2048]
    lin[b,h]  = h . fm_w[h]
    vx[b,h,r] = h . fm_V[h,r]
    diag[b,h] = (h*h) . (sum_r fm_V[h,r]^2)
    out[h,b]  = fm_w0[h] + lin + 0.5*(sum_r vx^2 - diag)

Device layout: activations kept as [feature_partition, batch_free] tiles so
every matmul contracts over the partition dim with natural-layout weights as
the stationary operand.  The FM stage flips to [batch_partition, col_free] by
using h^T k-tiles as the stationary operand.  All matmul inputs are bf16
(fp32 PSUM accumulation).

Schedule notes (v3).  The kernel head is HBM-bound: before L2 can run, x
(1 MiB) + W1 (2 MiB) + W2 (8 MiB) must land, ~44 us at the ~270 GB/s
effective per-core rate, while L1 only has ~28 us of compute to hide it.
Two structural moves recover the gap:

  * L2 and L3 each run as TWO k-half passes (kt 0-7, then kt 8-15).  Pass A
    drains its PSUM into a bf16 staging tile (zA); pass B accumulates the
    second half and a DVE add folds zA back into the PSUM before the tanh.
    L2-A therefore needs only W2's first half (~32 us) and starts right at
    L1-end (~40 us) instead of waiting for all of W2 (~47 us).  The split
    also relaxes every later weight-DMA deadline: W2's first-half tiles
    free at L2-A end (~95 us), so the pool-ring slots for W3's last tiles
    and VT open two layers early -- no just-in-time stalls at L3/FM.
  * L1 runs batch-chunk-outer (c0 jt0-15, then c1) so it starts once half
    of x and half of W1's columns land (~12 us); warm-up matmuls on a
    memset tile bridge the PE from engine-start so HAM stays un-throttled.

All weights are host-packed into [128, k*cols] order so each SBUF tile
fills with ONE contiguous DMA (~0.7 us sequencer cost per issue).  Output
is accumulated into one [128, 8*64] tile and shipped with a single DMA.
"""

import numpy as np
import ml_dtypes

import concourse.tile as tile
from concourse import bacc, mybir
from concourse import bass_utils

BF16 = mybir.dt.bfloat16
F32 = mybir.dt.float32
AF = mybir.ActivationFunctionType
ALU = mybir.AluOpType

P = 128
IN, HID, HEADS, RANK = 512, 2048, 64, 16
B = 8192
NCORES = 8
BC = B // NCORES            # 1024 batch rows per core
KT1 = IN // P               # 4  k-tiles, layer 1
KT = HID // P               # 16 k-tiles, layers 2/3 + FM
KH = KT // 2                # 8  k-tiles per half pass
JT = HID // P               # 16 output-feature tiles per layer
NB = 512                    # matmul moving free-dim (one PSUM bank)
NBC = BC // NB              # 2 batch column chunks
BT = BC // P                # 8 batch tiles in FM stage
HR = HEADS * RANK           # 1024 vx columns
WARMUP_MM = 10              # ~3.4us cold + ~0.5us warm of PE busy

_CACHE = {}


def _build_module():
    nc = bacc.Bacc(
        "TRN2", target_bir_lowering=False, debug=False, num_devices=NCORES
    )
    dt = nc.dram_tensor
    # host-packed layouts: [p, k*cols + c] = M[k*128 + p, c]
    XP = dt("XP", [P, KT1 * BC], BF16, kind="ExternalInput").ap()
    W1P = dt("W1P", [P, KT1 * HID], BF16, kind="ExternalInput").ap()
    W2P = dt("W2P", [P, KT * HID], BF16, kind="ExternalInput").ap()
    W3P = dt("W3P", [P, KT * HID], BF16, kind="ExternalInput").ap()
    VTP = dt("VTP", [P, KT * HR], BF16, kind="ExternalInput").ap()
    B1 = dt("B1", [P, JT], F32, kind="ExternalInput").ap()
    B2 = dt("B2", [P, JT], F32, kind="ExternalInput").ap()
    B3 = dt("B3", [P, JT], F32, kind="ExternalInput").ap()
    FW = dt("FW", [P, KT * HEADS], BF16, kind="ExternalInput").ap()
    SQ = dt("SQ", [P, KT * HEADS], BF16, kind="ExternalInput").ap()
    W0C = dt("W0C", [P, HEADS], BF16, kind="ExternalInput").ap()
    OUT = dt("out", [BC, HEADS], F32, kind="ExternalOutput").ap()

    with tile.TileContext(nc) as tc:
        with (
            tc.tile_pool(name="wpool", bufs=12) as wpool,   # 12 x 8KiB
            tc.tile_pool(name="hpool", bufs=32) as hpool,   # 32 x 2KiB
            tc.tile_pool(name="zpool", bufs=JT) as zpool,   # 16 x 2KiB staging
            tc.tile_pool(name="cpool", bufs=1) as cpool,
            tc.tile_pool(name="pp", bufs=8, space="PSUM") as pp,
            tc.tile_pool(name="epool", bufs=2) as epool,
            tc.tile_pool(name="spool", bufs=8) as spool,
            tc.tile_pool(name="opool", bufs=1) as opool,
        ):
            # --- PE warm-up --------------------------------------------------
            warm = cpool.tile([P, NB], BF16, tag="warm")
            nc.vector.memset(warm[:], 0.0)
            wu = pp.tile([P, NB], F32, tag="ps", name="warm")
            for _ in range(WARMUP_MM):
                nc.tensor.matmul(
                    wu[:], warm[:, 0:P], warm[:], start=True, stop=True
                )

            # --- SBUF tiles (allocation order defines pool-ring reuse) -------
            # wpool ring (12 slots): w3p[0:2] take fresh slots, w3p[2:4] the
            # W1 slots (free ~40us), w3p[4:8] the W2 pass-A slots (~95us),
            # vtq[0:4] the W2 pass-B slots (~150us).  All arrive well before
            # their first reads (L3-A ~151, L3-B ~207, FM ~262).
            xt = [hpool.tile([P, BC], BF16, tag="h", name=f"xt{k}")
                  for k in range(KT1)]
            w1p = [wpool.tile([P, 2 * HID], BF16, tag="w", name=f"w1p{k}")
                   for k in range(KT1 // 2)]
            w2p = [wpool.tile([P, 2 * HID], BF16, tag="w", name=f"w2p{k}")
                   for k in range(KT // 2)]
            w3p = [wpool.tile([P, 2 * HID], BF16, tag="w", name=f"w3p{k}")
                   for k in range(KT // 2)]
            vtq = [wpool.tile([P, 4 * HR], BF16, tag="w", name=f"vtq{q}")
                   for q in range(KT // 4)]
            zA = [zpool.tile([P, BC], BF16, tag="z", name=f"zA{j}")
                  for j in range(JT)]

            def wsl(tiles, kt, j):
                """[128,128] stationary slice for k-tile kt, out-feature j."""
                base = (kt % 2) * HID + j * P
                return tiles[kt // 2][:, base: base + P]

            # --- DMA issue plan (sync/gpsimd/scalar only, ~0.7us each) ------
            def w1_dma(k, h, eng):
                eng.dma_start(
                    w1p[k // 2][:, (k % 2) * HID + h * (HID // 2):
                                (k % 2) * HID + (h + 1) * (HID // 2)],
                    W1P[:, k * HID + h * (HID // 2): k * HID + (h + 1) * (HID // 2)],
                )

            # wave 1: x chunk c0 + W1 column-half h0 (enables L1 c0 jt0-7)
            w1_dma(0, 0, nc.scalar)
            nc.sync.dma_start(xt[0][:, 0:NB], XP[:, 0 * BC: 0 * BC + NB])
            nc.gpsimd.dma_start(xt[1][:, 0:NB], XP[:, 1 * BC: 1 * BC + NB])
            nc.scalar.dma_start(xt[2][:, 0:NB], XP[:, 2 * BC: 2 * BC + NB])
            nc.sync.dma_start(xt[3][:, 0:NB], XP[:, 3 * BC: 3 * BC + NB])
            w1_dma(1, 0, nc.sync)
            w1_dma(2, 0, nc.gpsimd)
            w1_dma(3, 0, nc.gpsimd)
            # wave 2: x c1 + W1 h1
            b1t = cpool.tile([P, JT], F32, tag="b1")
            nc.scalar.dma_start(b1t[:], B1)           # needed by first ACT
            nc.sync.dma_start(xt[0][:, NB:BC], XP[:, 0 * BC + NB: 1 * BC])
            nc.gpsimd.dma_start(xt[1][:, NB:BC], XP[:, 1 * BC + NB: 2 * BC])
            nc.scalar.dma_start(xt[2][:, NB:BC], XP[:, 2 * BC + NB: 3 * BC])
            nc.sync.dma_start(xt[3][:, NB:BC], XP[:, 3 * BC + NB: 4 * BC])
            w1_dma(0, 1, nc.scalar)
            w1_dma(1, 1, nc.sync)
            w1_dma(2, 1, nc.gpsimd)
            w1_dma(3, 1, nc.gpsimd)

            # W2 before anything else bulky (first half gates L2-A at ~40us)
            for j in range(4):
                nc.sync.dma_start(w2p[j][:], W2P[:, j * 2 * HID: (j + 1) * 2 * HID])

            b2t = cpool.tile([P, JT], F32, tag="b2")
            nc.gpsimd.dma_start(b2t[:], B2)
            b3t = cpool.tile([P, JT], F32, tag="b3")
            nc.gpsimd.dma_start(b3t[:], B3)
            # -w0/128 replicated; contracted against a ones column block so
            # the diag PSUM group finishes as (0.5*diag - w0).
            w0c = cpool.tile([P, HEADS], BF16, tag="w0c")
            nc.gpsimd.dma_start(w0c[:], W0C)
            onest = cpool.tile([P, P], BF16, tag="ones")
            nc.gpsimd.memset(onest[:], 1.0)
            fwt = cpool.tile([P, KT * HEADS], BF16, tag="fw")
            nc.gpsimd.dma_start(fwt[:], FW)
            sqt = cpool.tile([P, KT * HEADS], BF16, tag="sq")
            nc.gpsimd.dma_start(sqt[:], SQ)

            for j in range(4, 8):
                nc.gpsimd.dma_start(w2p[j][:], W2P[:, j * 2 * HID: (j + 1) * 2 * HID])
            for j in range(4):
                nc.sync.dma_start(w3p[j][:], W3P[:, j * 2 * HID: (j + 1) * 2 * HID])
            for j in range(4, 8):
                nc.gpsimd.dma_start(w3p[j][:], W3P[:, j * 2 * HID: (j + 1) * 2 * HID])
            nc.sync.dma_start(vtq[0][:], VTP[:, 0 * 4 * HR: 1 * 4 * HR])
            nc.sync.dma_start(vtq[1][:], VTP[:, 1 * 4 * HR: 2 * 4 * HR])
            nc.gpsimd.dma_start(vtq[2][:], VTP[:, 2 * 4 * HR: 3 * 4 * HR])
            nc.gpsimd.dma_start(vtq[3][:], VTP[:, 3 * 4 * HR: 4 * 4 * HR])

            # --- embedder ----------------------------------------------------
            def layer1():
                """c-outer so chunk c0 only needs wave-1 data."""
                h_out = [hpool.tile([P, BC], BF16, tag="h", name=f"l1h{j}")
                         for j in range(JT)]
                for c in range(NBC):
                    for jt in range(JT):
                        ps = pp.tile([P, NB], F32, tag="ps", name=f"l1ps{c}_{jt}")
                        kts = [(kt + jt) % KT1 for kt in range(KT1)]
                        for i, kt in enumerate(kts):
                            nc.tensor.matmul(
                                ps[:],
                                wsl(w1p, kt, jt),
                                xt[kt][:, c * NB: (c + 1) * NB],
                                start=(i == 0),
                                stop=(i == KT1 - 1),
                            )
                        nc.scalar.activation(
                            h_out[jt][:, c * NB: (c + 1) * NB],
                            ps[:],
                            AF.Tanh,
                            bias=b1t[:, jt: jt + 1],
                        )
                return h_out

            def layer2pass(h_prev, w_pairs, bias_t, name):
                """k-halved: pass A (kt 0-7) stages into bf16 zA, pass B
                (kt 8-15) accumulates, DVE folds zA in, ACT applies tanh."""
                # pass A
                for jt in range(JT):
                    ps = [pp.tile([P, NB], F32, tag="ps", name=f"{name}a{jt}_{c}")
                          for c in range(NBC)]
                    kts = [(kt + jt) % KH for kt in range(KH)]
                    for i, kt in enumerate(kts):
                        lhsT = wsl(w_pairs, kt, jt)
                        for c in range(NBC):
                            nc.tensor.matmul(
                                ps[c][:],
                                lhsT,
                                h_prev[kt][:, c * NB: (c + 1) * NB],
                                start=(i == 0),
                                stop=(i == KH - 1),
                            )
                    for c in range(NBC):
                        nc.vector.tensor_copy(
                            zA[jt][:, c * NB: (c + 1) * NB], ps[c][:]
                        )
                # pass B
                h_out = []
                for jt in range(JT):
                    ps = [pp.tile([P, NB], F32, tag="ps", name=f"{name}b{jt}_{c}")
                          for c in range(NBC)]
                    kts = [KH + (kt + jt) % KH for kt in range(KH)]
                    for i, kt in enumerate(kts):
                        lhsT = wsl(w_pairs, kt, jt)
                        for c in range(NBC):
                            nc.tensor.matmul(
                                ps[c][:],
                                lhsT,
                                h_prev[kt][:, c * NB: (c + 1) * NB],
                                start=(i == 0),
                                stop=(i == KH - 1),
                            )
                    ht = hpool.tile([P, BC], BF16, tag="h", name=f"{name}h{jt}")
                    for c in range(NBC):
                        nc.vector.tensor_add(
                            ps[c][:], ps[c][:], zA[jt][:, c * NB: (c + 1) * NB]
                        )
                        nc.scalar.activation(
                            ht[:, c * NB: (c + 1) * NB],
                            ps[c][:],
                            AF.Tanh,
                            bias=bias_t[:, jt: jt + 1],
                        )
                    h_out.append(ht)
                return h_out

            h1 = layer1()
            h2 = layer2pass(h1, w2p, b2t, "l2")
            h3 = layer2pass(h2, w3p, b3t, "l3")

            # --- h3 squared (stationary operand for the diag matmuls) -----
            h3sq = []
            for k in range(KT):
                sq_k = hpool.tile([P, BC], BF16, tag="h", name=f"h3sq{k}")
                nc.vector.tensor_mul(sq_k[:], h3[k][:], h3[k][:])
                h3sq.append(sq_k)

            # --- FM stage: per 128-row batch tile -------------------------
            def vsl(kt, half):
                """[128,512] moving slice of V^T for k-tile kt."""
                base = (kt % 4) * HR + half * NB
                return vtq[kt // 4][:, base: base + NB]

            def fm_phase_a(bt):
                """vx = h V^T (1024 cols) and lin = h fm_w^T (64 cols)."""
                vx0 = pp.tile([P, NB], F32, tag="ps", name=f"vx0_{bt}")
                vx1 = pp.tile([P, NB], F32, tag="ps", name=f"vx1_{bt}")
                lw = pp.tile([P, NB], F32, tag="ps", name=f"lw_{bt}")
                bsl = slice(bt * P, (bt + 1) * P)
                for kt in range(KT):
                    lhsT = h3[kt][:, bsl]
                    nc.tensor.matmul(
                        vx0[:], lhsT, vsl(kt, 0),
                        start=(kt == 0), stop=(kt == KT - 1),
                    )
                    nc.tensor.matmul(
                        vx1[:], lhsT, vsl(kt, 1),
                        start=(kt == 0), stop=(kt == KT - 1),
                    )
                    nc.tensor.matmul(
                        lw[:, 0:HEADS], lhsT,
                        fwt[:, kt * HEADS: (kt + 1) * HEADS],
                        start=(kt == 0), stop=(kt == KT - 1),
                    )
                return vx0, vx1, lw

            def fm_phase_b(bt):
                """diag = (h*h) . (0.5 * sum_r V^2), already scaled by 0.5."""
                dg = pp.tile([P, NB], F32, tag="ps", name=f"dg_{bt}")
                bsl = slice(bt * P, (bt + 1) * P)
                for kt in range(KT):
                    nc.tensor.matmul(
                        dg[:, 0:HEADS],
                        h3sq[kt][:, bsl],
                        sqt[:, kt * HEADS: (kt + 1) * HEADS],
                        start=(kt == 0), stop=False,
                    )
                nc.tensor.matmul(
                    dg[:, 0:HEADS], onest[:], w0c[:], start=False, stop=True,
                )
                return dg

            def fm_square_reduce(bt, vx0, vx1):
                """Emitted right after phase A: overlaps later bt's matmuls.
                Each 512-wide half squares then reduces independently so the
                two chains pipeline across ACT and DVE."""
                vx2 = epool.tile([P, HR], BF16, tag="e", name=f"vx2_{bt}")
                sumv = spool.tile([P, HEADS], F32, tag="s", name=f"sumv_{bt}")
                for c, vxh in ((0, vx0), (1, vx1)):
                    nc.scalar.activation(vx2[:, c * NB: (c + 1) * NB], vxh[:], AF.Square)
                    nc.vector.reduce_sum(
                        sumv[:, c * (HEADS // 2): (c + 1) * (HEADS // 2)],
                        vx2[:, c * NB: (c + 1) * NB].rearrange(
                            "p (h r) -> p h r", r=RANK
                        ),
                        axis=mybir.AxisListType.X,
                    )
                return sumv

            ot = opool.tile([P, BT * HEADS], F32, tag="o")

            def fm_combine(bt, sumv, lw, dg):
                # q = 0.5*sumv - diag_half
                q = spool.tile([P, HEADS], F32, tag="s", name=f"q_{bt}")
                nc.vector.scalar_tensor_tensor(
                    q[:], sumv[:], 0.5, dg[:, 0:HEADS],
                    op0=ALU.mult, op1=ALU.subtract,
                )
                nc.vector.tensor_add(
                    ot[:, bt * HEADS: (bt + 1) * HEADS], q[:], lw[:, 0:HEADS]
                )

            # Stagger: A(0), A(1), B(0), C(0), A(2), B(1), C(1), ...
            pend = []  # (bt, sumv, lw)
            for bt in range(BT):
                vx0, vx1, lw = fm_phase_a(bt)
                sumv = fm_square_reduce(bt, vx0, vx1)
                pend.append((bt, sumv, lw))
                if len(pend) == 2:
                    obt, osumv, olw = pend.pop(0)
                    dg = fm_phase_b(obt)
                    fm_combine(obt, osumv, olw, dg)
            while pend:
                obt, osumv, olw = pend.pop(0)
                dg = fm_phase_b(obt)
                fm_combine(obt, osumv, olw, dg)

            # one DMA for the whole per-core output: dram row bt*128+p.
            nc.gpsimd.dma_start(
                OUT.rearrange("(bt p) c -> p bt c", bt=BT), ot[:]
            )

    nc.compile()
    return nc


def _get_nc():
    if "nc" not in _CACHE:
        _CACHE["nc"] = _build_module()
    return _CACHE["nc"]


def _pack_rows(M, kt):
    """[kt*128, C] -> [128, kt*C] with [p, k*C+c] = M[k*128+p, c]."""
    C = M.shape[1]
    return np.ascontiguousarray(
        M.reshape(kt, P, C).transpose(1, 0, 2).reshape(P, kt * C)
    )


def _prep_host(x, W1, b1, W2, b2, W3, b3, fm_w0, fm_w, fm_V):
    """Host-side layout prep: bf16 casts, packing, per-head V reductions."""
    bf = ml_dtypes.bfloat16
    f32 = np.float32

    common = {
        "W1P": _pack_rows(W1.astype(bf), KT1),
        "W2P": _pack_rows(W2.astype(bf), KT),
        "W3P": _pack_rows(W3.astype(bf), KT),
        "B1": np.ascontiguousarray(b1.astype(f32).reshape(JT, P).T),
        "B2": np.ascontiguousarray(b2.astype(f32).reshape(JT, P).T),
        "B3": np.ascontiguousarray(b3.astype(f32).reshape(JT, P).T),
        # V^T: [2048, heads*rank] packed as [128, 16*1024]
        "VTP": _pack_rows(
            fm_V.reshape(HEADS * RANK, HID).T.astype(bf), KT
        ),
        # fm_w^T packed as [128, kt*64]: FW[p, kt*64+h] = fm_w[h, kt*128+p]
        "FW": np.ascontiguousarray(
            fm_w.T.reshape(KT, P, HEADS).transpose(1, 0, 2).reshape(P, KT * HEADS)
            .astype(bf)
        ),
        # 0.5 * sum_r V^2, same packing
        "SQ": np.ascontiguousarray(
            (0.5 * (fm_V.astype(np.float64) ** 2).sum(axis=1))
            .T.reshape(KT, P, HEADS).transpose(1, 0, 2).reshape(P, KT * HEADS)
            .astype(bf)
        ),
        "W0C": np.ascontiguousarray(
            np.tile((-fm_w0.astype(np.float64) / P)[None, :], (P, 1))
            .astype(ml_dtypes.bfloat16)
        ),
    }

    in_maps = []
    xb = x.astype(bf)
    for c in range(NCORES):
        m = dict(common)
        m["XP"] = _pack_rows(
            np.ascontiguousarray(xb[c * BC: (c + 1) * BC, :].T), KT1
        )
        in_maps.append(m)
    return in_maps


def kernel(x, W1, b1, W2, b2, W3, b3, fm_w0, fm_w, fm_V):
    # Host prep is plain numpy; coerce eagerly in case inputs are jax arrays.
    x, W1, b1, W2, b2, W3, b3, fm_w0, fm_w, fm_V = (
        np.asarray(a) for a in (x, W1, b1, W2, b2, W3, b3, fm_w0, fm_w, fm_V)
    )
    nc = _get_nc()
    in_maps = _prep_host(x, W1, b1, W2, b2, W3, b3, fm_w0, fm_w, fm_V)
    import os
    trace = bool(int(os.environ.get("KERNEL_TRACE", "0")))
    last_err = None
    for _attempt in range(3):
        try:
            res = bass_utils.run_bass_kernel_spmd(
                nc, in_maps, core_ids=list(range(NCORES)), trace=trace,
            )
            outs = [np.asarray(res.results[c]["out"]) for c in range(NCORES)]
            break
        except Exception as e:  # transient device faults (NRT unrecoverable)
            last_err = e
    else:
        raise last_err
    _CACHE["last_results"] = res
    full = np.concatenate(outs, axis=0)          # [B, HEADS]
    return np.ascontiguousarray(full.T).astype(np.float32)  # [HEADS, B]
